# revision 72
# baseline (speedup 1.0000x reference)
"""AdaptiveSpecAugment on 8 Trainium2 NeuronCores.

out[b,t,f] = 0 if (time_mask[b,t] or freq_mask[b,f]) else input_spec[b,t,f]

The masks come from jax threefry RNG with the fixed key 42, so the random
draws are replicated bit-exactly on host (tiny arrays); the device kernel
does the memory-bound streaming multiply
    out = in * keep_t[b,t] * keep_f[b,f]
with keep vectors in {0,1}.

Sharding: batch data-parallel, 32 samples per core. Per core, samples are
processed in groups of 4: 4*1600 = 6400 t-rows = 25 blocks of 256 rows
(2 consecutive rows per partition, so DMA descriptors move 640B contiguous
runs). keep_t is fed as a [128, 50] per-(partition, block, j) scalar table
broadcast along f via a stride-0 AP dim; keep_f as [128, 80] tiles
broadcast along blocks. The three blocks a sample boundary crosses use
host-premixed per-partition keep_f vectors so every multiply spans all 128
partitions. Loads stream on the sync HWDGE ring, stores on the scalar ring,
with an 8-deep tile pool for load/compute/store overlap.
"""

import os
import sys

import numpy as np

if "/opt/trn_rl_repo" not in sys.path and not any(
    p.endswith("trn_rl_repo") for p in sys.path
):
    sys.path.insert(0, "/opt/trn_rl_repo")

B, T, F = 256, 1600, 80
N_CORES = 8
BC = B // N_CORES          # samples per core = 32
S = 4                      # samples per group
NG = BC // S               # 8 groups per core
RPB = 256                  # rows per block
RJ = 2                     # rows per partition (j dim)
G = S * T // RPB           # 25 blocks per group
NKF = 2 * S - 1            # 4 pure + 3 boundary-mixed keep_f vectors per group
AUXW = G * RJ + NKF * F    # 50 + 560 = 610 floats per group
ROWS = S * T               # 6400 rows per group

# keep_f multiply spans per group: (block_lo, block_hi, kf_vector_index).
# Vectors 0..3 are the group's samples; 4..6 are partition-mixed vectors for
# the three blocks a sample boundary crosses (at partitions 32/64/96).
KF_SPANS = [(0, 6, 0), (6, 7, 4), (7, 12, 1), (12, 13, 5),
            (13, 18, 2), (18, 19, 6), (19, 25, 3)]
# boundary block -> (partition split, sample below, sample above)
KF_MIX = {4: (32, 0, 1), 5: (64, 1, 2), 6: (96, 2, 3)}


FREQ_MASKS = 2
TIME_MASKS = 0.05
FREQ_WIDTH = 27
TIME_WIDTH = 0.05
MAX_TIME_MASKS = 10


def _keep_vectors(length: np.ndarray):
    """Replicate the reference's _spec_masks RNG bit-exactly on host CPU.

    Returns keep_t [B,T] float32 and keep_f [B,F] float32 (1.0 = keep).
    """
    import jax
    import jax.numpy as jnp

    cpu = jax.local_devices(backend="cpu")[0]
    with jax.default_device(cpu):
        length = jnp.asarray(np.asarray(length))
        key = jax.random.key(42)
        kf1, kf2, kt1, kt2 = jax.random.split(key, 4)

        x_left = jax.random.randint(kf1, (B, FREQ_MASKS), 0, F - FREQ_WIDTH + 1)
        w_f = jax.random.randint(kf2, (B, FREQ_MASKS), 0, FREQ_WIDTH + 1)
        f_idx = jnp.arange(F)
        freq_mask = jnp.any(
            (f_idx >= x_left[..., None]) & (f_idx < (x_left + w_f)[..., None]),
            axis=1,
        )  # [B, F]

        lf = length.astype(jnp.float32)
        n_t = jnp.minimum(MAX_TIME_MASKS, (lf * TIME_MASKS).astype(jnp.int32))
        tw = jnp.maximum(1, (lf * TIME_WIDTH).astype(jnp.int32))
        hi = jnp.maximum(1, length - tw)

        u1 = jax.random.uniform(kt1, (B, MAX_TIME_MASKS))
        y_left = jnp.minimum(
            (u1 * (hi[:, None] + 1).astype(jnp.float32)).astype(jnp.int32),
            hi[:, None],
        )
        u2 = jax.random.uniform(kt2, (B, MAX_TIME_MASKS))
        w_t = jnp.minimum(
            (u2 * (tw[:, None] + 1).astype(jnp.float32)).astype(jnp.int32),
            tw[:, None],
        )
        active = jnp.arange(MAX_TIME_MASKS)[None, :] < n_t[:, None]
        t_idx = jnp.arange(T)
        time_mask = jnp.any(
            active[..., None]
            & (t_idx >= y_left[..., None])
            & (t_idx < (y_left + w_t)[..., None]),
            axis=1,
        )  # [B, T]

        keep_t = 1.0 - np.asarray(time_mask, dtype=np.float32)
        keep_f = 1.0 - np.asarray(freq_mask, dtype=np.float32)
    return keep_t, keep_f


def _units(grp):
    """Pipeline units (block ranges) for a group: 13/12-block halves, with
    the very first and last units shrunk to reduce kernel head/tail latency."""
    GH = (G + 1) // 2
    if grp == 0:
        return [(0, 4), (4, GH), (GH, G)]
    if grp == NG - 1:
        return [(0, GH), (GH, G - 4), (G - 4, G)]
    return [(0, GH), (GH, G)]


_NC_CACHE = {}


def _all_units():
    """Flat (grp, b_lo, b_hi) pipeline unit list across the whole core."""
    return [(grp, b_lo, b_hi) for grp in range(NG) for b_lo, b_hi in _units(grp)]


def _build_bass_raw():
    """Hand-scheduled raw-bass version: one wait per instruction, no Tile
    event-semaphore machinery, minimal preamble/epilogue."""
    import concourse.bass as bass
    from concourse import mybir

    f32 = mybir.dt.float32
    nc = bass.Bass()
    x = nc.declare_dram_parameter("x", [BC, T, F], f32, isOutput=False)
    # aux2 columns: kt tables | mixed kf vectors | pure kf vectors
    KTW = NG * G * RJ                      # 400
    MIXW = NG * len(KF_MIX) * F            # 1920
    PUREW = NG * S * F                     # 2560
    aux2 = nc.declare_dram_parameter(
        "aux2", [128, KTW + MIXW + PUREW], f32, isOutput=False
    )
    y = nc.declare_dram_parameter("y", [BC, T, F], f32, isOutput=True)

    xv = x[:].flatten_outer_dims().rearrange(
        "(grp g p j) f -> grp g p j f", g=G, p=128, j=RJ
    )
    yv = y[:].flatten_outer_dims().rearrange(
        "(grp g p j) f -> grp g p j f", g=G, p=128, j=RJ
    )

    units = _all_units()
    NU = len(units)
    K = 8                 # data slots (double-buffer depth)
    NL = 8                # semaphore lanes for load/store completion
    SLOT = ((G + 1) // 2) * RJ * F  # 2080 floats per slot

    from contextlib import ExitStack

    with ExitStack() as ctx:
        block = ctx.enter_context(nc.Block())
        ld = [ctx.enter_context(nc.semaphore(f"ld{i}")) for i in range(NL)]
        st = [ctx.enter_context(nc.semaphore(f"st{i}")) for i in range(NL)]
        cm = ctx.enter_context(nc.semaphore("cm"))
        ax = ctx.enter_context(nc.semaphore("ax"))
        dslab = ctx.enter_context(nc.sbuf_tensor("dslab", [128, K * SLOT], f32))
        aux2t = ctx.enter_context(
            nc.sbuf_tensor("aux2t", [128, KTW + MIXW + PUREW], f32)
        )

        def slot_ap(u, nb):
            return dslab[:, (u % K) * SLOT : (u % K) * SLOT + nb * RJ * F].rearrange(
                "p (g j f) -> p g j f", j=RJ, f=F
            )

        @block.gpsimd
        def _(g):
            g.dma_start(out=aux2t[:], in_=aux2[:]).then_inc(ax, 16)

        @block.sync
        def _(sp):
            for u, (grp, b_lo, b_hi) in enumerate(units):
                if u >= K:
                    v = u - K  # store v must have completed to reuse the slot
                    sp.wait_ge(st[v % NL], 16 * (v // NL + 1))
                sp.dma_start(
                    out=slot_ap(u, b_hi - b_lo),
                    in_=xv[grp][b_lo:b_hi].rearrange("g p j f -> p g j f"),
                ).then_inc(ld[u % NL], 16)

        @block.vector
        def _(v):
            v.wait_ge(ax, 16)
            for u, (grp, b_lo, b_hi) in enumerate(units):
                nb = b_hi - b_lo
                d4 = slot_ap(u, nb)
                v.wait_ge(ld[u % NL], 16 * (u // NL + 1))
                kt = (
                    aux2t[:, grp * G * RJ + b_lo * RJ : grp * G * RJ + b_hi * RJ]
                    .rearrange("p (g j) -> p g j", j=RJ)
                    .unsqueeze(3)
                    .broadcast_to([128, nb, RJ, F])
                )
                last = v.tensor_mul(d4, d4, kt)
                for g_lo, g_hi, kfi in KF_SPANS:
                    g_lo, g_hi = max(g_lo, b_lo), min(g_hi, b_hi)
                    if g_lo >= g_hi:
                        continue
                    if kfi < S:
                        o = KTW + MIXW + (grp * S + kfi) * F
                    else:
                        o = KTW + (grp * len(KF_MIX) + (kfi - S)) * F
                    kf = aux2t[:, o : o + F]
                    gl, gh = g_lo - b_lo, g_hi - b_lo
                    dst = d4[:, gl:gh, :, :]
                    src = kf.unsqueeze(1).unsqueeze(2).broadcast_to(
                        [128, gh - gl, RJ, F]
                    )
                    last = v.tensor_mul(dst, dst, src)
                last.then_inc(cm, 1)

        @block.scalar
        def _(sc):
            for u, (grp, b_lo, b_hi) in enumerate(units):
                sc.wait_ge(cm, u + 1)
                sc.dma_start(
                    out=yv[grp][b_lo:b_hi].rearrange("g p j f -> p g j f"),
                    in_=slot_ap(u, b_hi - b_lo),
                ).then_inc(st[u % NL], 16)
            # drain: all stores landed before the NEFF completes
            for i in range(NL):
                n = len([u for u in range(NU) if u % NL == i])
                sc.wait_ge(st[i], 16 * n)

    if not nc.is_finalized():
        nc.finalize()
    return nc


def _build_bass():
    import concourse.bass as bass
    import concourse.tile as tile
    from concourse import bacc, mybir

    f32 = mybir.dt.float32
    nc = bacc.Bacc()
    x = nc.declare_dram_parameter("x", [BC, T, F], f32, isOutput=False)
    # per-group aux block: kt table [50] + 7 keep_f vectors (4 pure + 3 mixed)
    aux2 = nc.declare_dram_parameter("aux2", [128, NG * AUXW], f32, isOutput=False)
    y = nc.declare_dram_parameter("y", [BC, T, F], f32, isOutput=True)

    # [group, block, partition, j, f] view of the contiguous shard
    xv = x[:].flatten_outer_dims().rearrange(
        "(grp g p j) f -> grp g p j f", g=G, p=128, j=RJ
    )
    yv = y[:].flatten_outer_dims().rearrange(
        "(grp g p j) f -> grp g p j f", g=G, p=128, j=RJ
    )

    n_units = sum(len(_units(grp)) for grp in range(NG))

    with tile.TileContext(nc) as tc:
        with (
            tc.tile_pool(name="data", bufs=DATA_BUFS) as data,
            tc.tile_pool(name="auxp", bufs=1) as auxp,
        ):
            # per-group aux DMAs so early units aren't gated on the full table
            aux_all = auxp.tile([128, NG * AUXW], f32)
            for grp in range(NG):
                sl = slice(grp * AUXW, (grp + 1) * AUXW)
                nc.scalar.dma_start(out=aux_all[:, sl], in_=aux2[:, sl])
            uidx = 0
            for grp in range(NG):
                ktg = aux_all[:, grp * AUXW : grp * AUXW + G * RJ]

                for b_lo, b_hi in _units(grp):
                    nb = b_hi - b_lo
                    dtile = data.tile([128, nb * RJ * F], f32, tag="d")
                    d4 = dtile[:].rearrange("p (g j f) -> p g j f", j=RJ, f=F)
                    nc.sync.dma_start(
                        out=d4,
                        in_=xv[grp][b_lo:b_hi].rearrange("g p j f -> p g j f"),
                    )

                    kt = (
                        ktg[:, b_lo * RJ : b_hi * RJ]
                        .rearrange("p (g j) -> p g j", j=RJ)
                        .unsqueeze(3)
                        .broadcast_to([128, nb, RJ, F])
                    )
                    nc.vector.tensor_mul(d4, d4, kt)

                    for g_lo, g_hi, kfi in KF_SPANS:
                        g_lo, g_hi = max(g_lo, b_lo), min(g_hi, b_hi)
                        if g_lo >= g_hi:
                            continue
                        o = grp * AUXW + G * RJ + kfi * F
                        kf = aux_all[:, o : o + F]
                        gl, gh = g_lo - b_lo, g_hi - b_lo
                        dst = d4[:, gl:gh, :, :]
                        src = (
                            kf.unsqueeze(1)
                            .unsqueeze(2)
                            .broadcast_to([128, gh - gl, RJ, F])
                        )
                        nc.vector.tensor_mul(dst, dst, src)

                    # near the end the sync ring is done loading, so spread the
                    # tail stores across both HWDGE rings; the very last
                    # (smallest) store goes on scalar, which drains first
                    st_eng = (
                        nc.sync
                        if n_units - 4 <= uidx < n_units - 1
                        else nc.scalar
                    )
                    st_eng.dma_start(
                        out=yv[grp][b_lo:b_hi].rearrange("g p j f -> p g j f"),
                        in_=d4,
                    )
                    uidx += 1
    if not nc.is_finalized():
        nc.finalize()
    return nc


IMPL = os.environ.get("KERNEL_IMPL", "tile")
DATA_BUFS = int(os.environ.get("KERNEL_BUFS", "8"))


def _get_nc():
    if "nc" not in _NC_CACHE:
        _NC_CACHE["nc"] = _build_bass_raw() if IMPL == "raw" else _build_bass()
    return _NC_CACHE["nc"]


def _kt_mix_tables(keep_t: np.ndarray, keep_f: np.ndarray):
    """Per-core kt tables [c, grp, 128, 50] and mixed kf vectors
    [c, grp, 3, 128, F]."""
    kt = keep_t.reshape(N_CORES, NG, G, 128, RJ).transpose(0, 1, 3, 2, 4)
    kt = kt.reshape(N_CORES, NG, 128, G * RJ)
    kf = keep_f.reshape(N_CORES, NG, S, F)
    p_idx = np.arange(128)
    mix = np.empty((N_CORES, NG, len(KF_MIX), 128, F), np.float32)
    for mi, kfi in enumerate(sorted(KF_MIX)):
        p_split, s_lo, s_hi = KF_MIX[kfi]
        mix[:, :, mi] = np.where(
            (p_idx < p_split)[None, None, :, None],
            kf[:, :, s_lo][:, :, None, :],
            kf[:, :, s_hi][:, :, None, :],
        )
    return kt, mix


def _pack_aux(keep_t: np.ndarray, keep_f: np.ndarray):
    """Tile-variant aux: per-group [kt 50 | pure kf 320 | mixed kf 240]."""
    kt, mix = _kt_mix_tables(keep_t, keep_f)
    kf = keep_f.reshape(N_CORES, NG, S, F)
    aux = np.empty((N_CORES, NG, 128, AUXW), np.float32)
    aux[..., : G * RJ] = kt
    for s in range(S):
        aux[..., G * RJ + s * F : G * RJ + (s + 1) * F] = kf[:, :, s][:, :, None, :]
    mixp = mix.transpose(0, 1, 3, 2, 4)  # [c, grp, 128, 3, F]
    aux[..., G * RJ + S * F :] = mixp.reshape(N_CORES, NG, 128, len(KF_MIX) * F)
    return np.ascontiguousarray(aux.transpose(0, 2, 1, 3)).reshape(
        N_CORES, 128, NG * AUXW
    )


def _pack_aux_raw(keep_t: np.ndarray, keep_f: np.ndarray):
    kt, mix = _kt_mix_tables(keep_t, keep_f)
    akt = np.ascontiguousarray(kt.transpose(0, 2, 1, 3)).reshape(
        N_CORES, 128, NG * G * RJ
    )
    mixp = np.ascontiguousarray(mix.transpose(0, 3, 1, 2, 4)).reshape(
        N_CORES, 128, -1
    )
    purep = np.broadcast_to(
        keep_f.reshape(N_CORES, 1, NG * S * F), (N_CORES, 128, NG * S * F)
    )
    return np.ascontiguousarray(np.concatenate([akt, mixp, purep], axis=2))


LAST_RESULTS = None


def kernel(input_spec: np.ndarray, length: np.ndarray):
    global LAST_RESULTS
    from concourse.bass_utils import run_bass_kernel_spmd

    input_spec = np.asarray(input_spec, dtype=np.float32)
    length = np.asarray(length, dtype=np.int32)
    assert input_spec.shape == (B, T, F), input_spec.shape

    keep_t, keep_f = _keep_vectors(length)
    if IMPL == "raw":
        aux2 = _pack_aux_raw(keep_t, keep_f)
        in_maps = [
            {
                "x": np.ascontiguousarray(input_spec[c * BC : (c + 1) * BC]),
                "aux2": aux2[c],
            }
            for c in range(N_CORES)
        ]
    else:
        aux2 = _pack_aux(keep_t, keep_f)
        in_maps = [
            {
                "x": np.ascontiguousarray(input_spec[c * BC : (c + 1) * BC]),
                "aux2": aux2[c],
            }
            for c in range(N_CORES)
        ]

    nc = _get_nc()
    last_err = None
    for attempt in range(3):
        try:
            res = run_bass_kernel_spmd(nc, in_maps, core_ids=list(range(N_CORES)))
            break
        except Exception as e:  # transient NRT/device errors: retry
            last_err = e
            import time

            time.sleep(2.0 * (attempt + 1))
    else:
        raise last_err
    LAST_RESULTS = res
    out = np.concatenate([res.results[c]["y"] for c in range(N_CORES)], axis=0)
    return out, length


# revision 73
# speedup vs baseline: 1.0387x; 1.0387x over previous
"""AdaptiveSpecAugment on 8 Trainium2 NeuronCores.

out[b,t,f] = 0 if (time_mask[b,t] or freq_mask[b,f]) else input_spec[b,t,f]

The masks come from jax threefry RNG with the fixed key 42, so the random
draws are replicated bit-exactly on host (tiny arrays); the device kernel
does the memory-bound streaming multiply
    out = in * keep_t[b,t] * keep_f[b,f]
with keep vectors in {0,1}.

Sharding: batch data-parallel, 32 samples per core. Per core, samples are
processed in groups of 4: 4*1600 = 6400 t-rows = 25 blocks of 256 rows
(2 consecutive rows per partition, so DMA descriptors move 640B contiguous
runs). keep_t is fed as a [128, 50] per-(partition, block, j) scalar table
broadcast along f via a stride-0 AP dim; keep_f as [128, 80] tiles
broadcast along blocks. The three blocks a sample boundary crosses use
host-premixed per-partition keep_f vectors so every multiply spans all 128
partitions. Loads stream on the sync HWDGE ring, stores on the scalar ring,
with an 8-deep tile pool for load/compute/store overlap.
"""

import os
import sys

import numpy as np

if "/opt/trn_rl_repo" not in sys.path and not any(
    p.endswith("trn_rl_repo") for p in sys.path
):
    sys.path.insert(0, "/opt/trn_rl_repo")

B, T, F = 256, 1600, 80
N_CORES = 8
BC = B // N_CORES          # samples per core = 32
S = 4                      # samples per group
NG = BC // S               # 8 groups per core
RPB = 256                  # rows per block
RJ = 2                     # rows per partition (j dim)
G = S * T // RPB           # 25 blocks per group
NKF = 2 * S - 1            # 4 pure + 3 boundary-mixed keep_f vectors per group
AUXW = G * RJ + NKF * F    # 50 + 560 = 610 floats per group
ROWS = S * T               # 6400 rows per group

# keep_f multiply spans per group: (block_lo, block_hi, kf_vector_index).
# Vectors 0..3 are the group's samples; 4..6 are partition-mixed vectors for
# the three blocks a sample boundary crosses (at partitions 32/64/96).
KF_SPANS = [(0, 6, 0), (6, 7, 4), (7, 12, 1), (12, 13, 5),
            (13, 18, 2), (18, 19, 6), (19, 25, 3)]
# boundary block -> (partition split, sample below, sample above)
KF_MIX = {4: (32, 0, 1), 5: (64, 1, 2), 6: (96, 2, 3)}


FREQ_MASKS = 2
TIME_MASKS = 0.05
FREQ_WIDTH = 27
TIME_WIDTH = 0.05
MAX_TIME_MASKS = 10


def _keep_vectors(length: np.ndarray):
    """Replicate the reference's _spec_masks RNG bit-exactly on host CPU.

    Returns keep_t [B,T] float32 and keep_f [B,F] float32 (1.0 = keep).
    """
    import jax
    import jax.numpy as jnp

    cpu = jax.local_devices(backend="cpu")[0]
    with jax.default_device(cpu):
        length = jnp.asarray(np.asarray(length))
        key = jax.random.key(42)
        kf1, kf2, kt1, kt2 = jax.random.split(key, 4)

        x_left = jax.random.randint(kf1, (B, FREQ_MASKS), 0, F - FREQ_WIDTH + 1)
        w_f = jax.random.randint(kf2, (B, FREQ_MASKS), 0, FREQ_WIDTH + 1)
        f_idx = jnp.arange(F)
        freq_mask = jnp.any(
            (f_idx >= x_left[..., None]) & (f_idx < (x_left + w_f)[..., None]),
            axis=1,
        )  # [B, F]

        lf = length.astype(jnp.float32)
        n_t = jnp.minimum(MAX_TIME_MASKS, (lf * TIME_MASKS).astype(jnp.int32))
        tw = jnp.maximum(1, (lf * TIME_WIDTH).astype(jnp.int32))
        hi = jnp.maximum(1, length - tw)

        u1 = jax.random.uniform(kt1, (B, MAX_TIME_MASKS))
        y_left = jnp.minimum(
            (u1 * (hi[:, None] + 1).astype(jnp.float32)).astype(jnp.int32),
            hi[:, None],
        )
        u2 = jax.random.uniform(kt2, (B, MAX_TIME_MASKS))
        w_t = jnp.minimum(
            (u2 * (tw[:, None] + 1).astype(jnp.float32)).astype(jnp.int32),
            tw[:, None],
        )
        active = jnp.arange(MAX_TIME_MASKS)[None, :] < n_t[:, None]
        t_idx = jnp.arange(T)
        time_mask = jnp.any(
            active[..., None]
            & (t_idx >= y_left[..., None])
            & (t_idx < (y_left + w_t)[..., None]),
            axis=1,
        )  # [B, T]

        keep_t = 1.0 - np.asarray(time_mask, dtype=np.float32)
        keep_f = 1.0 - np.asarray(freq_mask, dtype=np.float32)
    return keep_t, keep_f


def _units(grp):
    """Pipeline units (block ranges) for a group: 13/12-block halves, with
    the very first and last units shrunk to reduce kernel head/tail latency."""
    GH = (G + 1) // 2
    if grp == 0:
        return [(0, 4), (4, GH), (GH, G)]
    if grp == NG - 1:
        return [(0, GH), (GH, G - 4), (G - 4, G)]
    return [(0, GH), (GH, G)]


_NC_CACHE = {}


def _all_units():
    """Flat (grp, b_lo, b_hi) pipeline unit list across the whole core."""
    return [(grp, b_lo, b_hi) for grp in range(NG) for b_lo, b_hi in _units(grp)]


def _build_bass_raw():
    """Hand-scheduled raw-bass version: one wait per instruction, no Tile
    event-semaphore machinery, minimal preamble/epilogue."""
    import concourse.bass as bass
    from concourse import mybir

    f32 = mybir.dt.float32
    nc = bass.Bass()
    x = nc.declare_dram_parameter("x", [BC, T, F], f32, isOutput=False)
    # aux2 columns: kt tables | mixed kf vectors | pure kf vectors
    KTW = NG * G * RJ                      # 400
    MIXW = NG * len(KF_MIX) * F            # 1920
    PUREW = NG * S * F                     # 2560
    aux2 = nc.declare_dram_parameter(
        "aux2", [128, KTW + MIXW + PUREW], f32, isOutput=False
    )
    y = nc.declare_dram_parameter("y", [BC, T, F], f32, isOutput=True)

    xv = x[:].flatten_outer_dims().rearrange(
        "(grp g p j) f -> grp g p j f", g=G, p=128, j=RJ
    )
    yv = y[:].flatten_outer_dims().rearrange(
        "(grp g p j) f -> grp g p j f", g=G, p=128, j=RJ
    )

    units = _all_units()
    NU = len(units)
    K = 8                 # data slots (double-buffer depth)
    NL = 8                # semaphore lanes for load/store completion
    SLOT = ((G + 1) // 2) * RJ * F  # 2080 floats per slot

    from contextlib import ExitStack

    with ExitStack() as ctx:
        block = ctx.enter_context(nc.Block())
        ld = [ctx.enter_context(nc.semaphore(f"ld{i}")) for i in range(NL)]
        st = [ctx.enter_context(nc.semaphore(f"st{i}")) for i in range(NL)]
        cm = ctx.enter_context(nc.semaphore("cm"))
        ax = ctx.enter_context(nc.semaphore("ax"))
        dslab = ctx.enter_context(nc.sbuf_tensor("dslab", [128, K * SLOT], f32))
        aux2t = ctx.enter_context(
            nc.sbuf_tensor("aux2t", [128, KTW + MIXW + PUREW], f32)
        )

        def slot_ap(u, nb):
            return dslab[:, (u % K) * SLOT : (u % K) * SLOT + nb * RJ * F].rearrange(
                "p (g j f) -> p g j f", j=RJ, f=F
            )

        @block.gpsimd
        def _(g):
            g.dma_start(out=aux2t[:], in_=aux2[:]).then_inc(ax, 16)

        @block.sync
        def _(sp):
            for u, (grp, b_lo, b_hi) in enumerate(units):
                if u >= K:
                    v = u - K  # store v must have completed to reuse the slot
                    sp.wait_ge(st[v % NL], 16 * (v // NL + 1))
                sp.dma_start(
                    out=slot_ap(u, b_hi - b_lo),
                    in_=xv[grp][b_lo:b_hi].rearrange("g p j f -> p g j f"),
                ).then_inc(ld[u % NL], 16)

        @block.vector
        def _(v):
            v.wait_ge(ax, 16)
            for u, (grp, b_lo, b_hi) in enumerate(units):
                nb = b_hi - b_lo
                d4 = slot_ap(u, nb)
                v.wait_ge(ld[u % NL], 16 * (u // NL + 1))
                kt = (
                    aux2t[:, grp * G * RJ + b_lo * RJ : grp * G * RJ + b_hi * RJ]
                    .rearrange("p (g j) -> p g j", j=RJ)
                    .unsqueeze(3)
                    .broadcast_to([128, nb, RJ, F])
                )
                last = v.tensor_mul(d4, d4, kt)
                for g_lo, g_hi, kfi in KF_SPANS:
                    g_lo, g_hi = max(g_lo, b_lo), min(g_hi, b_hi)
                    if g_lo >= g_hi:
                        continue
                    if kfi < S:
                        o = KTW + MIXW + (grp * S + kfi) * F
                    else:
                        o = KTW + (grp * len(KF_MIX) + (kfi - S)) * F
                    kf = aux2t[:, o : o + F]
                    gl, gh = g_lo - b_lo, g_hi - b_lo
                    dst = d4[:, gl:gh, :, :]
                    src = kf.unsqueeze(1).unsqueeze(2).broadcast_to(
                        [128, gh - gl, RJ, F]
                    )
                    last = v.tensor_mul(dst, dst, src)
                last.then_inc(cm, 1)

        @block.scalar
        def _(sc):
            for u, (grp, b_lo, b_hi) in enumerate(units):
                sc.wait_ge(cm, u + 1)
                sc.dma_start(
                    out=yv[grp][b_lo:b_hi].rearrange("g p j f -> p g j f"),
                    in_=slot_ap(u, b_hi - b_lo),
                ).then_inc(st[u % NL], 16)
            # drain: all stores landed before the NEFF completes
            for i in range(NL):
                n = len([u for u in range(NU) if u % NL == i])
                sc.wait_ge(st[i], 16 * n)

    if not nc.is_finalized():
        nc.finalize()
    return nc


def _build_bass():
    import concourse.bass as bass
    import concourse.tile as tile
    from concourse import bacc, mybir

    f32 = mybir.dt.float32
    nc = bacc.Bacc()
    x = nc.declare_dram_parameter("x", [BC, T, F], f32, isOutput=False)
    # per-group aux block: kt table [50] + 7 keep_f vectors (4 pure + 3 mixed)
    aux2 = nc.declare_dram_parameter("aux2", [128, NG * AUXW], f32, isOutput=False)
    y = nc.declare_dram_parameter("y", [BC, T, F], f32, isOutput=True)

    # [group, block, partition, j, f] view of the contiguous shard
    xv = x[:].flatten_outer_dims().rearrange(
        "(grp g p j) f -> grp g p j f", g=G, p=128, j=RJ
    )
    yv = y[:].flatten_outer_dims().rearrange(
        "(grp g p j) f -> grp g p j f", g=G, p=128, j=RJ
    )

    n_units = sum(len(_units(grp)) for grp in range(NG))

    with tile.TileContext(nc) as tc:
        with (
            tc.tile_pool(name="data", bufs=DATA_BUFS) as data,
            tc.tile_pool(name="auxp", bufs=1) as auxp,
        ):
            # per-group aux DMAs so early units aren't gated on the full table
            aux_all = auxp.tile([128, NG * AUXW], f32)
            for grp in range(NG):
                sl = slice(grp * AUXW, (grp + 1) * AUXW)
                nc.scalar.dma_start(out=aux_all[:, sl], in_=aux2[:, sl])
            uidx = 0
            for grp in range(NG):
                ktg = aux_all[:, grp * AUXW : grp * AUXW + G * RJ]

                for b_lo, b_hi in _units(grp):
                    nb = b_hi - b_lo
                    dtile = data.tile([128, nb * RJ * F], f32, tag="d")
                    d4 = dtile[:].rearrange("p (g j f) -> p g j f", j=RJ, f=F)
                    nc.sync.dma_start(
                        out=d4,
                        in_=xv[grp][b_lo:b_hi].rearrange("g p j f -> p g j f"),
                    )

                    kt = (
                        ktg[:, b_lo * RJ : b_hi * RJ]
                        .rearrange("p (g j) -> p g j", j=RJ)
                        .unsqueeze(3)
                        .broadcast_to([128, nb, RJ, F])
                    )
                    nc.vector.tensor_mul(d4, d4, kt)

                    for g_lo, g_hi, kfi in KF_SPANS:
                        g_lo, g_hi = max(g_lo, b_lo), min(g_hi, b_hi)
                        if g_lo >= g_hi:
                            continue
                        o = grp * AUXW + G * RJ + kfi * F
                        kf = aux_all[:, o : o + F]
                        gl, gh = g_lo - b_lo, g_hi - b_lo
                        dst = d4[:, gl:gh, :, :]
                        src = (
                            kf.unsqueeze(1)
                            .unsqueeze(2)
                            .broadcast_to([128, gh - gl, RJ, F])
                        )
                        nc.vector.tensor_mul(dst, dst, src)

                    # near the end the sync ring is done loading, so spread the
                    # tail stores across both HWDGE rings; the very last
                    # (smallest) store goes on scalar, which drains first
                    st_eng = (
                        nc.sync
                        if n_units - 3 <= uidx < n_units - 1
                        else nc.scalar
                    )
                    st_eng.dma_start(
                        out=yv[grp][b_lo:b_hi].rearrange("g p j f -> p g j f"),
                        in_=d4,
                    )
                    uidx += 1
    if not nc.is_finalized():
        nc.finalize()
    return nc


IMPL = os.environ.get("KERNEL_IMPL", "tile")
DATA_BUFS = int(os.environ.get("KERNEL_BUFS", "8"))


def _get_nc():
    if "nc" not in _NC_CACHE:
        _NC_CACHE["nc"] = _build_bass_raw() if IMPL == "raw" else _build_bass()
    return _NC_CACHE["nc"]


def _kt_mix_tables(keep_t: np.ndarray, keep_f: np.ndarray):
    """Per-core kt tables [c, grp, 128, 50] and mixed kf vectors
    [c, grp, 3, 128, F]."""
    kt = keep_t.reshape(N_CORES, NG, G, 128, RJ).transpose(0, 1, 3, 2, 4)
    kt = kt.reshape(N_CORES, NG, 128, G * RJ)
    kf = keep_f.reshape(N_CORES, NG, S, F)
    p_idx = np.arange(128)
    mix = np.empty((N_CORES, NG, len(KF_MIX), 128, F), np.float32)
    for mi, kfi in enumerate(sorted(KF_MIX)):
        p_split, s_lo, s_hi = KF_MIX[kfi]
        mix[:, :, mi] = np.where(
            (p_idx < p_split)[None, None, :, None],
            kf[:, :, s_lo][:, :, None, :],
            kf[:, :, s_hi][:, :, None, :],
        )
    return kt, mix


def _pack_aux(keep_t: np.ndarray, keep_f: np.ndarray):
    """Tile-variant aux: per-group [kt 50 | pure kf 320 | mixed kf 240]."""
    kt, mix = _kt_mix_tables(keep_t, keep_f)
    kf = keep_f.reshape(N_CORES, NG, S, F)
    aux = np.empty((N_CORES, NG, 128, AUXW), np.float32)
    aux[..., : G * RJ] = kt
    for s in range(S):
        aux[..., G * RJ + s * F : G * RJ + (s + 1) * F] = kf[:, :, s][:, :, None, :]
    mixp = mix.transpose(0, 1, 3, 2, 4)  # [c, grp, 128, 3, F]
    aux[..., G * RJ + S * F :] = mixp.reshape(N_CORES, NG, 128, len(KF_MIX) * F)
    return np.ascontiguousarray(aux.transpose(0, 2, 1, 3)).reshape(
        N_CORES, 128, NG * AUXW
    )


def _pack_aux_raw(keep_t: np.ndarray, keep_f: np.ndarray):
    kt, mix = _kt_mix_tables(keep_t, keep_f)
    akt = np.ascontiguousarray(kt.transpose(0, 2, 1, 3)).reshape(
        N_CORES, 128, NG * G * RJ
    )
    mixp = np.ascontiguousarray(mix.transpose(0, 3, 1, 2, 4)).reshape(
        N_CORES, 128, -1
    )
    purep = np.broadcast_to(
        keep_f.reshape(N_CORES, 1, NG * S * F), (N_CORES, 128, NG * S * F)
    )
    return np.ascontiguousarray(np.concatenate([akt, mixp, purep], axis=2))


LAST_RESULTS = None


def kernel(input_spec: np.ndarray, length: np.ndarray):
    global LAST_RESULTS
    from concourse.bass_utils import run_bass_kernel_spmd

    input_spec = np.asarray(input_spec, dtype=np.float32)
    length = np.asarray(length, dtype=np.int32)
    assert input_spec.shape == (B, T, F), input_spec.shape

    keep_t, keep_f = _keep_vectors(length)
    if IMPL == "raw":
        aux2 = _pack_aux_raw(keep_t, keep_f)
        in_maps = [
            {
                "x": np.ascontiguousarray(input_spec[c * BC : (c + 1) * BC]),
                "aux2": aux2[c],
            }
            for c in range(N_CORES)
        ]
    else:
        aux2 = _pack_aux(keep_t, keep_f)
        in_maps = [
            {
                "x": np.ascontiguousarray(input_spec[c * BC : (c + 1) * BC]),
                "aux2": aux2[c],
            }
            for c in range(N_CORES)
        ]

    nc = _get_nc()
    last_err = None
    for attempt in range(3):
        try:
            res = run_bass_kernel_spmd(nc, in_maps, core_ids=list(range(N_CORES)))
            break
        except Exception as e:  # transient NRT/device errors: retry
            last_err = e
            import time

            time.sleep(2.0 * (attempt + 1))
    else:
        raise last_err
    LAST_RESULTS = res
    out = np.concatenate([res.results[c]["y"] for c in range(N_CORES)], axis=0)
    return out, length


# revision 74
# speedup vs baseline: 1.1390x; 1.0966x over previous
"""AdaptiveSpecAugment on 8 Trainium2 NeuronCores.

out[b,t,f] = 0 if (time_mask[b,t] or freq_mask[b,f]) else input_spec[b,t,f]

The masks come from jax threefry RNG with the fixed key 42, so the random
draws are replicated bit-exactly on host (tiny arrays); the device kernel
does the memory-bound streaming multiply
    out = in * keep_t[b,t] * keep_f[b,f]
with keep vectors in {0,1}.

Sharding: batch data-parallel, 32 samples per core. Per core, samples are
processed in groups of 4: 4*1600 = 6400 t-rows = 25 blocks of 256 rows
(2 consecutive rows per partition, so DMA descriptors move 640B contiguous
runs). keep_t is fed as a [128, 50] per-(partition, block, j) scalar table
broadcast along f via a stride-0 AP dim; keep_f as [128, 80] tiles
broadcast along blocks. The three blocks a sample boundary crosses use
host-premixed per-partition keep_f vectors so every multiply spans all 128
partitions. Loads stream on the sync HWDGE ring, stores on the scalar ring,
with an 8-deep tile pool for load/compute/store overlap.
"""

import os
import sys

import numpy as np

if "/opt/trn_rl_repo" not in sys.path and not any(
    p.endswith("trn_rl_repo") for p in sys.path
):
    sys.path.insert(0, "/opt/trn_rl_repo")

B, T, F = 256, 1600, 80
N_CORES = 8
BC = B // N_CORES          # samples per core = 32
S = 4                      # samples per group
NG = BC // S               # 8 groups per core
RPB = 256                  # rows per block
RJ = 2                     # rows per partition (j dim)
G = S * T // RPB           # 25 blocks per group
NKF = 2 * S - 1            # 4 pure + 3 boundary-mixed keep_f vectors per group
AUXW = G * RJ + NKF * F    # 50 + 560 = 610 floats per group
ROWS = S * T               # 6400 rows per group

# keep_f multiply spans per group: (block_lo, block_hi, kf_vector_index).
# Vectors 0..3 are the group's samples; 4..6 are partition-mixed vectors for
# the three blocks a sample boundary crosses (at partitions 32/64/96).
KF_SPANS = [(0, 6, 0), (6, 7, 4), (7, 12, 1), (12, 13, 5),
            (13, 18, 2), (18, 19, 6), (19, 25, 3)]
# boundary block -> (partition split, sample below, sample above)
KF_MIX = {4: (32, 0, 1), 5: (64, 1, 2), 6: (96, 2, 3)}


FREQ_MASKS = 2
TIME_MASKS = 0.05
FREQ_WIDTH = 27
TIME_WIDTH = 0.05
MAX_TIME_MASKS = 10


def _keep_vectors(length: np.ndarray):
    """Replicate the reference's _spec_masks RNG bit-exactly on host CPU.

    Returns keep_t [B,T] float32 and keep_f [B,F] float32 (1.0 = keep).
    """
    import jax
    import jax.numpy as jnp

    cpu = jax.local_devices(backend="cpu")[0]
    with jax.default_device(cpu):
        length = jnp.asarray(np.asarray(length))
        key = jax.random.key(42)
        kf1, kf2, kt1, kt2 = jax.random.split(key, 4)

        x_left = jax.random.randint(kf1, (B, FREQ_MASKS), 0, F - FREQ_WIDTH + 1)
        w_f = jax.random.randint(kf2, (B, FREQ_MASKS), 0, FREQ_WIDTH + 1)
        f_idx = jnp.arange(F)
        freq_mask = jnp.any(
            (f_idx >= x_left[..., None]) & (f_idx < (x_left + w_f)[..., None]),
            axis=1,
        )  # [B, F]

        lf = length.astype(jnp.float32)
        n_t = jnp.minimum(MAX_TIME_MASKS, (lf * TIME_MASKS).astype(jnp.int32))
        tw = jnp.maximum(1, (lf * TIME_WIDTH).astype(jnp.int32))
        hi = jnp.maximum(1, length - tw)

        u1 = jax.random.uniform(kt1, (B, MAX_TIME_MASKS))
        y_left = jnp.minimum(
            (u1 * (hi[:, None] + 1).astype(jnp.float32)).astype(jnp.int32),
            hi[:, None],
        )
        u2 = jax.random.uniform(kt2, (B, MAX_TIME_MASKS))
        w_t = jnp.minimum(
            (u2 * (tw[:, None] + 1).astype(jnp.float32)).astype(jnp.int32),
            tw[:, None],
        )
        active = jnp.arange(MAX_TIME_MASKS)[None, :] < n_t[:, None]
        t_idx = jnp.arange(T)
        time_mask = jnp.any(
            active[..., None]
            & (t_idx >= y_left[..., None])
            & (t_idx < (y_left + w_t)[..., None]),
            axis=1,
        )  # [B, T]

        keep_t = 1.0 - np.asarray(time_mask, dtype=np.float32)
        keep_f = 1.0 - np.asarray(freq_mask, dtype=np.float32)
    return keep_t, keep_f


def _units(grp):
    """Pipeline units (block ranges) for a group: 13/12-block halves, with
    the very first and last units shrunk to reduce kernel head/tail latency."""
    GH = (G + 1) // 2
    if grp == 0:
        return [(0, 4), (4, GH), (GH, G)]
    if grp == NG - 1:
        return [(0, GH), (GH, G - 4), (G - 4, G)]
    return [(0, GH), (GH, G)]


_NC_CACHE = {}


def _all_units():
    """Flat (grp, b_lo, b_hi) pipeline unit list across the whole core."""
    return [(grp, b_lo, b_hi) for grp in range(NG) for b_lo, b_hi in _units(grp)]


def _build_bass_raw():
    """Hand-scheduled raw-bass version: one wait per instruction, no Tile
    event-semaphore machinery, minimal preamble/epilogue."""
    import concourse.bass as bass
    from concourse import mybir

    f32 = mybir.dt.float32
    nc = bass.Bass()
    x = nc.declare_dram_parameter("x", [BC, T, F], f32, isOutput=False)
    # aux2 columns: kt tables | mixed kf vectors | pure kf vectors
    KTW = NG * G * RJ                      # 400
    MIXW = NG * len(KF_MIX) * F            # 1920
    PUREW = NG * S * F                     # 2560
    aux2 = nc.declare_dram_parameter(
        "aux2", [128, KTW + MIXW + PUREW], f32, isOutput=False
    )
    y = nc.declare_dram_parameter("y", [BC, T, F], f32, isOutput=True)

    xv = x[:].flatten_outer_dims().rearrange(
        "(grp g p j) f -> grp g p j f", g=G, p=128, j=RJ
    )
    yv = y[:].flatten_outer_dims().rearrange(
        "(grp g p j) f -> grp g p j f", g=G, p=128, j=RJ
    )

    units = _all_units()
    NU = len(units)
    K = 8                 # data slots (double-buffer depth)
    NL = 8                # semaphore lanes for load/store completion
    SLOT = ((G + 1) // 2) * RJ * F  # 2080 floats per slot

    from contextlib import ExitStack

    with ExitStack() as ctx:
        block = ctx.enter_context(nc.Block())
        ld = [ctx.enter_context(nc.semaphore(f"ld{i}")) for i in range(NL)]
        st = [ctx.enter_context(nc.semaphore(f"st{i}")) for i in range(NL)]
        cm = ctx.enter_context(nc.semaphore("cm"))
        ax = ctx.enter_context(nc.semaphore("ax"))
        dslab = ctx.enter_context(nc.sbuf_tensor("dslab", [128, K * SLOT], f32))
        aux2t = ctx.enter_context(
            nc.sbuf_tensor("aux2t", [128, KTW + MIXW + PUREW], f32)
        )

        def slot_ap(u, nb):
            return dslab[:, (u % K) * SLOT : (u % K) * SLOT + nb * RJ * F].rearrange(
                "p (g j f) -> p g j f", j=RJ, f=F
            )

        @block.gpsimd
        def _(g):
            g.dma_start(out=aux2t[:], in_=aux2[:]).then_inc(ax, 16)

        @block.sync
        def _(sp):
            for u, (grp, b_lo, b_hi) in enumerate(units):
                if u >= K:
                    v = u - K  # store v must have completed to reuse the slot
                    sp.wait_ge(st[v % NL], 16 * (v // NL + 1))
                sp.dma_start(
                    out=slot_ap(u, b_hi - b_lo),
                    in_=xv[grp][b_lo:b_hi].rearrange("g p j f -> p g j f"),
                ).then_inc(ld[u % NL], 16)

        @block.vector
        def _(v):
            v.wait_ge(ax, 16)
            for u, (grp, b_lo, b_hi) in enumerate(units):
                nb = b_hi - b_lo
                d4 = slot_ap(u, nb)
                v.wait_ge(ld[u % NL], 16 * (u // NL + 1))
                kt = (
                    aux2t[:, grp * G * RJ + b_lo * RJ : grp * G * RJ + b_hi * RJ]
                    .rearrange("p (g j) -> p g j", j=RJ)
                    .unsqueeze(3)
                    .broadcast_to([128, nb, RJ, F])
                )
                last = v.tensor_mul(d4, d4, kt)
                for g_lo, g_hi, kfi in KF_SPANS:
                    g_lo, g_hi = max(g_lo, b_lo), min(g_hi, b_hi)
                    if g_lo >= g_hi:
                        continue
                    if kfi < S:
                        o = KTW + MIXW + (grp * S + kfi) * F
                    else:
                        o = KTW + (grp * len(KF_MIX) + (kfi - S)) * F
                    kf = aux2t[:, o : o + F]
                    gl, gh = g_lo - b_lo, g_hi - b_lo
                    dst = d4[:, gl:gh, :, :]
                    src = kf.unsqueeze(1).unsqueeze(2).broadcast_to(
                        [128, gh - gl, RJ, F]
                    )
                    last = v.tensor_mul(dst, dst, src)
                last.then_inc(cm, 1)

        @block.scalar
        def _(sc):
            for u, (grp, b_lo, b_hi) in enumerate(units):
                sc.wait_ge(cm, u + 1)
                sc.dma_start(
                    out=yv[grp][b_lo:b_hi].rearrange("g p j f -> p g j f"),
                    in_=slot_ap(u, b_hi - b_lo),
                ).then_inc(st[u % NL], 16)
            # drain: all stores landed before the NEFF completes
            for i in range(NL):
                n = len([u for u in range(NU) if u % NL == i])
                sc.wait_ge(st[i], 16 * n)

    if not nc.is_finalized():
        nc.finalize()
    return nc


def _build_bass():
    import concourse.bass as bass
    import concourse.tile as tile
    from concourse import bacc, mybir

    f32 = mybir.dt.float32
    nc = bacc.Bacc()
    x = nc.declare_dram_parameter("x", [BC, T, F], f32, isOutput=False)
    # per-group aux block: kt table [50] + 7 keep_f vectors (4 pure + 3 mixed)
    aux2 = nc.declare_dram_parameter("aux2", [128, NG * AUXW], f32, isOutput=False)
    y = nc.declare_dram_parameter("y", [BC, T, F], f32, isOutput=True)

    # [group, block, partition, j, f] view of the contiguous shard
    xv = x[:].flatten_outer_dims().rearrange(
        "(grp g p j) f -> grp g p j f", g=G, p=128, j=RJ
    )
    yv = y[:].flatten_outer_dims().rearrange(
        "(grp g p j) f -> grp g p j f", g=G, p=128, j=RJ
    )

    n_units = sum(len(_units(grp)) for grp in range(NG))

    with tile.TileContext(nc) as tc:
        with (
            tc.tile_pool(name="data", bufs=DATA_BUFS) as data,
            tc.tile_pool(name="auxp", bufs=1) as auxp,
        ):
            # per-group aux DMAs so early units aren't gated on the full table
            aux_all = auxp.tile([128, NG * AUXW], f32)
            for grp in range(NG):
                sl = slice(grp * AUXW, (grp + 1) * AUXW)
                nc.scalar.dma_start(out=aux_all[:, sl], in_=aux2[:, sl])
            uidx = 0
            for grp in range(NG):
                ktg = aux_all[:, grp * AUXW : grp * AUXW + G * RJ]

                for b_lo, b_hi in _units(grp):
                    nb = b_hi - b_lo
                    dtile = data.tile([128, nb * RJ * F], f32, tag="d")
                    d4 = dtile[:].rearrange("p (g j f) -> p g j f", j=RJ, f=F)
                    nc.sync.dma_start(
                        out=d4,
                        in_=xv[grp][b_lo:b_hi].rearrange("g p j f -> p g j f"),
                    )

                    kt = (
                        ktg[:, b_lo * RJ : b_hi * RJ]
                        .rearrange("p (g j) -> p g j", j=RJ)
                        .unsqueeze(3)
                        .broadcast_to([128, nb, RJ, F])
                    )
                    nc.vector.tensor_mul(d4, d4, kt)

                    for g_lo, g_hi, kfi in KF_SPANS:
                        g_lo, g_hi = max(g_lo, b_lo), min(g_hi, b_hi)
                        if g_lo >= g_hi:
                            continue
                        o = grp * AUXW + G * RJ + kfi * F
                        kf = aux_all[:, o : o + F]
                        gl, gh = g_lo - b_lo, g_hi - b_lo
                        dst = d4[:, gl:gh, :, :]
                        src = (
                            kf.unsqueeze(1)
                            .unsqueeze(2)
                            .broadcast_to([128, gh - gl, RJ, F])
                        )
                        nc.vector.tensor_mul(dst, dst, src)

                    # near the end the sync ring is done loading, so spread the
                    # tail stores across both HWDGE rings; the very last
                    # (smallest) store goes on scalar, which drains first
                    st_eng = nc.sync if uidx >= n_units - 3 else nc.scalar
                    st_eng.dma_start(
                        out=yv[grp][b_lo:b_hi].rearrange("g p j f -> p g j f"),
                        in_=d4,
                    )
                    uidx += 1
    if not nc.is_finalized():
        nc.finalize()
    return nc


IMPL = os.environ.get("KERNEL_IMPL", "tile")
DATA_BUFS = int(os.environ.get("KERNEL_BUFS", "8"))


def _get_nc():
    if "nc" not in _NC_CACHE:
        _NC_CACHE["nc"] = _build_bass_raw() if IMPL == "raw" else _build_bass()
    return _NC_CACHE["nc"]


def _kt_mix_tables(keep_t: np.ndarray, keep_f: np.ndarray):
    """Per-core kt tables [c, grp, 128, 50] and mixed kf vectors
    [c, grp, 3, 128, F]."""
    kt = keep_t.reshape(N_CORES, NG, G, 128, RJ).transpose(0, 1, 3, 2, 4)
    kt = kt.reshape(N_CORES, NG, 128, G * RJ)
    kf = keep_f.reshape(N_CORES, NG, S, F)
    p_idx = np.arange(128)
    mix = np.empty((N_CORES, NG, len(KF_MIX), 128, F), np.float32)
    for mi, kfi in enumerate(sorted(KF_MIX)):
        p_split, s_lo, s_hi = KF_MIX[kfi]
        mix[:, :, mi] = np.where(
            (p_idx < p_split)[None, None, :, None],
            kf[:, :, s_lo][:, :, None, :],
            kf[:, :, s_hi][:, :, None, :],
        )
    return kt, mix


def _pack_aux(keep_t: np.ndarray, keep_f: np.ndarray):
    """Tile-variant aux: per-group [kt 50 | pure kf 320 | mixed kf 240]."""
    kt, mix = _kt_mix_tables(keep_t, keep_f)
    kf = keep_f.reshape(N_CORES, NG, S, F)
    aux = np.empty((N_CORES, NG, 128, AUXW), np.float32)
    aux[..., : G * RJ] = kt
    for s in range(S):
        aux[..., G * RJ + s * F : G * RJ + (s + 1) * F] = kf[:, :, s][:, :, None, :]
    mixp = mix.transpose(0, 1, 3, 2, 4)  # [c, grp, 128, 3, F]
    aux[..., G * RJ + S * F :] = mixp.reshape(N_CORES, NG, 128, len(KF_MIX) * F)
    return np.ascontiguousarray(aux.transpose(0, 2, 1, 3)).reshape(
        N_CORES, 128, NG * AUXW
    )


def _pack_aux_raw(keep_t: np.ndarray, keep_f: np.ndarray):
    kt, mix = _kt_mix_tables(keep_t, keep_f)
    akt = np.ascontiguousarray(kt.transpose(0, 2, 1, 3)).reshape(
        N_CORES, 128, NG * G * RJ
    )
    mixp = np.ascontiguousarray(mix.transpose(0, 3, 1, 2, 4)).reshape(
        N_CORES, 128, -1
    )
    purep = np.broadcast_to(
        keep_f.reshape(N_CORES, 1, NG * S * F), (N_CORES, 128, NG * S * F)
    )
    return np.ascontiguousarray(np.concatenate([akt, mixp, purep], axis=2))


LAST_RESULTS = None


def kernel(input_spec: np.ndarray, length: np.ndarray):
    global LAST_RESULTS
    from concourse.bass_utils import run_bass_kernel_spmd

    input_spec = np.asarray(input_spec, dtype=np.float32)
    length = np.asarray(length, dtype=np.int32)
    assert input_spec.shape == (B, T, F), input_spec.shape

    keep_t, keep_f = _keep_vectors(length)
    if IMPL == "raw":
        aux2 = _pack_aux_raw(keep_t, keep_f)
        in_maps = [
            {
                "x": np.ascontiguousarray(input_spec[c * BC : (c + 1) * BC]),
                "aux2": aux2[c],
            }
            for c in range(N_CORES)
        ]
    else:
        aux2 = _pack_aux(keep_t, keep_f)
        in_maps = [
            {
                "x": np.ascontiguousarray(input_spec[c * BC : (c + 1) * BC]),
                "aux2": aux2[c],
            }
            for c in range(N_CORES)
        ]

    nc = _get_nc()
    last_err = None
    for attempt in range(3):
        try:
            res = run_bass_kernel_spmd(nc, in_maps, core_ids=list(range(N_CORES)))
            break
        except Exception as e:  # transient NRT/device errors: retry
            last_err = e
            import time

            time.sleep(2.0 * (attempt + 1))
    else:
        raise last_err
    LAST_RESULTS = res
    out = np.concatenate([res.results[c]["y"] for c in range(N_CORES)], axis=0)
    return out, length
